# revision 1
# baseline (speedup 1.0000x reference)
"""Confusion-matrix kernel for Trainium2 (8 NeuronCores, data-parallel over batch).

Per batch b (one per core):
    pred[n]  = argmax_c input[b, c, n]            (n = pixel, N = H*W)
    cm[i, j] = sum_n target[b, i, n] * (pred[n] == j)
    rs[i]    = sum_n target[b, i, n]
Host: cm_b = cm / (rs + 1e-8); out = mean_b cm_b.

Device layout: pixel-major [N, C] tiles ([128 partitions, K groups, 21]).
Per 128-pixel group: one-hot of argmax built via reduce_max + is_ge (DVE),
then matmul lhsT=target[128,21], rhs=onehot_ext[128,22] accumulated into a
single [21,22] f32 PSUM tile (column 21 is all-ones -> row sums).

Raw-bass pipeline (double buffered, NT tiles):
    SP  : x-load t   (gated on sv >= 2t       : is_ge(t-2) freed x slot)
    ACT : y-load t   (gated on sp >= t-1      : matmuls(t-2) freed y slot)
    DVE : reduce_max -> is_ge(t)  (gated on sx>=16(t+1), sp>=t-1 for h slot)
    PE  : 256 matmuls(t)          (gated on sv>=4+2t, sy>=16(t+1))
"""

from contextlib import ExitStack

import ml_dtypes
import numpy as np

import concourse.bass as bass
import concourse.mybir as mybir
from concourse.bass_utils import run_bass_kernel_spmd

B, C, H, W = 8, 21, 512, 512
N = H * W            # 262144 pixels per batch
P = 128              # SBUF partitions
K = 128              # pixel-groups per SBUF tile (each group = 128 pixels)
NT = N // (P * K)    # 8 outer tiles per core
CE = C + 1           # one-hot columns + ones column (row-sum accumulator)
N_CORES = 8

_CACHED_NC = None


def build_nc():
    nc = bass.Bass()
    x = nc.declare_dram_parameter("x", [NT, P, K * C], mybir.dt.float16, isOutput=False)
    y = nc.declare_dram_parameter("y", [NT, P, K * C], mybir.dt.float16, isOutput=False)
    out = nc.declare_dram_parameter("out", [C, CE], mybir.dt.float32, isOutput=True)

    with ExitStack() as ctx:
        xs = [
            ctx.enter_context(nc.sbuf_tensor(f"xsb{i}", [P, K * C], mybir.dt.float16))
            for i in range(2)
        ]
        ys = [
            ctx.enter_context(nc.sbuf_tensor(f"ysb{i}", [P, K * C], mybir.dt.float16))
            for i in range(2)
        ]
        hs = [
            ctx.enter_context(nc.sbuf_tensor(f"hsb{i}", [P, K * CE], mybir.dt.float16))
            for i in range(2)
        ]
        ms = [
            ctx.enter_context(nc.sbuf_tensor(f"msb{i}", [P, K], mybir.dt.float16))
            for i in range(2)
        ]
        ot = ctx.enter_context(nc.sbuf_tensor("otsb", [C, CE], mybir.dt.float32))
        cm_psum = ctx.enter_context(nc.psum_tensor("cmps", [C, CE], mybir.dt.float32))

        block = ctx.enter_context(nc.Block())
        sxs = [ctx.enter_context(nc.semaphore(f"sx{i}")) for i in range(2)]
        sys_ = [ctx.enter_context(nc.semaphore(f"sy{i}")) for i in range(2)]
        sv = ctx.enter_context(nc.semaphore("sv"))
        sp = ctx.enter_context(nc.semaphore("sp"))
        so = ctx.enter_context(nc.semaphore("so"))

        @block.sync
        def _(sync):
            for t in range(NT):
                if t >= 2:
                    # x slot freed once is_ge(t-2) consumed it
                    sync.wait_ge(sv, 2 * t)
                sync.dma_start(out=xs[t % 2][:], in_=x[t]).then_inc(sxs[t % 2], 16)
            sync.wait_ge(sv, 2 * NT + 3)
            sync.dma_start(out=out[:], in_=ot[:]).then_inc(so, 16)
            sync.wait_ge(so, 16)

        @block.scalar
        def _(scalar):
            for t in range(NT):
                if t >= 2:
                    # y slot freed once matmuls(t-2) consumed it
                    scalar.wait_ge(sp, t - 1)
                scalar.dma_start(out=ys[t % 2][:], in_=y[t]).then_inc(sys_[t % 2], 16)

        @block.vector
        def _(vector):
            for ht in hs:
                h3 = ht[:].rearrange("p (k c) -> p k c", c=CE)
                nc.vector.memset(h3[:, :, C:CE], 1.0).then_inc(sv, 1)  # sv: 1, 2
            for t in range(NT):
                xt = xs[t % 2]
                ht = hs[t % 2]
                mt = ms[t % 2]
                x3 = xt[:].rearrange("p (k c) -> p k c", c=C)
                h3 = ht[:].rearrange("p (k c) -> p k c", c=CE)
                vector.wait_ge(sxs[t % 2], 16 * (t // 2 + 1))
                nc.vector.reduce_max(
                    mt[:], x3, axis=mybir.AxisListType.X
                ).then_inc(sv, 1)  # sv = 3 + 2t
                vector.wait_ge(sv, 3 + 2 * t)   # reduce_max(t) retired (same-engine RAW)
                if t >= 2:
                    # h slot freed once matmuls(t-2) consumed it
                    vector.wait_ge(sp, t - 1)
                nc.vector.tensor_tensor(
                    out=h3[:, :, 0:C],
                    in0=x3,
                    in1=mt[:].unsqueeze(2).to_broadcast((P, K, C)),
                    op=mybir.AluOpType.is_ge,
                ).then_inc(sv, 1)  # sv = 4 + 2t
            vector.wait_ge(sp, NT)
            nc.vector.tensor_copy(ot[:], cm_psum[:]).then_inc(sv, 1)  # sv = 2NT+3

        @block.tensor
        def _(tensor):
            for t in range(NT):
                yt = ys[t % 2]
                ht = hs[t % 2]
                tensor.wait_ge(sv, 4 + 2 * t)   # is_ge(t) done (implies ones cols)
                tensor.wait_ge(sys_[t % 2], 16 * (t // 2 + 1))
                for k in range(K):
                    mm = nc.tensor.matmul(
                        out=cm_psum[:],
                        lhsT=yt[:, k * C : (k + 1) * C],
                        rhs=ht[:, k * CE : (k + 1) * CE],
                        start=(t == 0 and k == 0),
                        stop=(t == NT - 1 and k == K - 1),
                    )
                mm.then_inc(sp, 1)  # sp = t + 1

    return nc


def _get_nc():
    global _CACHED_NC
    if _CACHED_NC is None:
        _CACHED_NC = build_nc()
    return _CACHED_NC


def make_in_maps(input, target):
    inp = np.asarray(input, dtype=np.float32)
    tgt = np.asarray(target, dtype=np.float32)
    in_maps = []
    for b in range(B):
        xb = np.ascontiguousarray(inp[b].reshape(C, N).T).astype(np.float16)
        yb = np.ascontiguousarray(tgt[b].reshape(C, N).T).astype(np.float16)
        in_maps.append(
            {
                "x": xb.reshape(NT, P, K * C),
                "y": yb.reshape(NT, P, K * C),
            }
        )
    return in_maps


def postprocess(outs):
    acc = np.stack([np.asarray(o, dtype=np.float64) for o in outs])  # [B, C, CE]
    cm = acc[:, :, :C] / (acc[:, :, C:] + 1e-8)
    return cm.mean(axis=0).astype(np.float32)


def kernel(input, target):
    nc = _get_nc()
    in_maps = make_in_maps(input, target)
    res = run_bass_kernel_spmd(nc, in_maps, list(range(N_CORES)))
    return postprocess([r["out"] for r in res.results])

